# revision 1
# baseline (speedup 1.0000x reference)
"""Trainium2 Bass kernel for nn_LogicLayer (soft logic-gate layer).

Math (per core, batch-sharded):
  pA = softmax(Wa, axis=1); pB = softmax(Wb, axis=1); pT = softmax(tw, axis=0)
  a = pA @ X ; b = pB @ X
  out = sum_g pT[g] * gate_g(a, b)

Each of the 16 soft gates is affine in {1, A, B, A*B}, so with C[g, :] =
(c1, cA, cB, cAB) per gate:
  out = w1 + wA*a + wB*b + wAB*(a*b),   w_j[m] = sum_g pT[g, m] * C[g, j]

All softmax normalizers fold into the coefficients: with unnormalized
Ea = exp(Wa) (no max-subtraction needed; Wa ~ N(0,1)), ta = Ea^T-matmul,
a = ta / sA[m], and pT = exp(tw)/sT:
  out = w1' + wA'*ta + wB'*tb + wAB'*ta*tb
  w1' = w1raw/sT, wA' = wAraw/(sT*sA), wB' = wBraw/(sT*sB),
  wAB' = wABraw/(sT*sA*sB)

Device pipeline (per core; batch 16384 sharded 8 ways -> NB=2048):
  1. Load Wa^T, Wb^T (host-staged transposed layout), exp -> bf16 EaT/EbT.
  2. Row sums sA[m] via tiny N=1 PE matmuls against a ones vector.
  3. exp(tw) f32; one [16,5] constant matmul gives all 4 raw coefficient
     combos + sT; small DVE ops assemble final per-row coefficients.
  4. Main loop: 2 bf16 matmul groups (K=1024) per (m-block, n-tile 512),
     fused epilogue on DVE/ACT/GPSIMD, DMA out.
"""

import sys

if "/opt/trn_rl_repo" not in sys.path:
    sys.path.insert(0, "/opt/trn_rl_repo")

import numpy as np

import concourse.bass as bass
import concourse.mybir as mybir
import concourse.tile as tile
from concourse.bass_utils import run_bass_kernel_spmd

N_CORES = 8
SIZE = 1024
PREV = 1024
BATCH = 16384
NB = BATCH // N_CORES  # 2048 batch columns per core
NT = 512               # n-tile (one PSUM bank of f32)
N_NT = NB // NT        # 4
KB = PREV // 128       # 8 k-blocks
MB = SIZE // 128       # 8 m-blocks

F32 = mybir.dt.float32
BF16 = mybir.dt.bfloat16
FP8 = mybir.dt.float8e4

# fp8e4m3 + DoubleRow: ~1.5x PE throughput on the two big matmuls at the
# cost of ~2.5e-3 max rel err (CPU-sim; bf16 gives ~2e-4).
USE_FP8 = False

# Gate coefficient matrix: columns = [const, A, B, AB, ones]; rows = gate id.
_C16 = np.array(
    [
        # 1   A   B  AB  ones
        [0,  0,  0,  0, 1],  # 0  FALSE
        [0,  0,  0,  1, 1],  # 1  A AND B
        [0,  1,  0, -1, 1],  # 2  A AND NOT B
        [0,  1,  0,  0, 1],  # 3  A
        [0,  0,  1, -1, 1],  # 4  NOT A AND B
        [0,  0,  1,  0, 1],  # 5  B
        [0,  1,  1, -2, 1],  # 6  XOR
        [0,  1,  1, -1, 1],  # 7  OR
        [1, -1, -1,  1, 1],  # 8  NOR
        [1, -1, -1,  2, 1],  # 9  XNOR
        [1,  0, -1,  0, 1],  # 10 NOT B
        [1,  0, -1,  1, 1],  # 11 B -> A
        [1, -1,  0,  0, 1],  # 12 NOT A
        [1, -1,  0,  1, 1],  # 13 A -> B
        [1,  0,  0, -1, 1],  # 14 NAND
        [1,  0,  0,  0, 1],  # 15 TRUE
    ],
    dtype=np.float32,
)


def _split_waits(nc, maxw=1):
    """Walrus in this container encodes at most one sync-wait per
    instruction; hoist excess waits into preceding NoOps on the same
    engine (semantically an AND of waits, executed in sequence)."""
    for f in nc.m.functions:
        for blk in f.blocks:
            new_list = []
            changed = False
            for inst in blk.instructions:
                si = inst.sync_info
                if si is not None and len(si.on_wait) > maxw:
                    waits = list(si.on_wait)
                    chunks = [waits[i : i + maxw] for i in range(0, len(waits), maxw)]
                    for ci, ch in enumerate(chunks[:-1]):
                        nop = mybir.InstNoOp(
                            name=f"{inst.name}-wsplit{ci}", ins=[], outs=[]
                        )
                        nop.engine = inst.engine
                        nop.sync_info = mybir.SyncInfo(on_wait=ch, on_update=[])
                        new_list.append(nop)
                    inst.sync_info = mybir.SyncInfo(
                        on_wait=chunks[-1], on_update=list(si.on_update)
                    )
                    changed = True
                new_list.append(inst)
            if changed:
                blk.instructions = new_list


def build_nc(reps=1):
    # reps>1 repeats the main loop inside the NEFF (timing only: slope
    # between two reps values isolates steady-state main-loop time from
    # the ~8ms axon dispatch floor).
    nc = bass.Bass()
    x_d = nc.dram_tensor("x", [PREV, NB], F32, kind="ExternalInput")
    wat_d = nc.dram_tensor("wat", [PREV, SIZE], F32, kind="ExternalInput")
    wbt_d = nc.dram_tensor("wbt", [PREV, SIZE], F32, kind="ExternalInput")
    tw_d = nc.dram_tensor("tw", [16, SIZE], F32, kind="ExternalInput")
    out_d = nc.dram_tensor("out", [SIZE, NB], F32, kind="ExternalOutput")
    c16_d = nc.inline_tensor(_C16, "c16")

    AF = mybir.ActivationFunctionType
    OP = mybir.AluOpType

    with tile.TileContext(nc) as tc:
        with (
            tc.tile_pool(name="persist", bufs=1) as pp,
            tc.tile_pool(name="wstage", bufs=3) as wstage,
            tc.tile_pool(name="xstage", bufs=6) as xstage,
            tc.tile_pool(name="xbuf", bufs=2) as xbuf,
            tc.tile_pool(name="epi", bufs=3) as epi,
            tc.tile_pool(name="outp", bufs=4) as outp,
            tc.tile_pool(name="psum", bufs=2, space="PSUM") as psp,
            tc.tile_pool(name="psum1", bufs=1, space="PSUM") as psp1,
        ):
            # --- constants ---
            c16s = pp.tile([16, 5], F32, tag="c16s", name="c16s")
            nc.sync.dma_start(out=c16s, in_=c16_d[:, :])
            ones = pp.tile([128, 1], BF16, tag="ones", name="ones")
            nc.vector.memset(ones, 1.0)

            # --- table coefficients ---
            tws = pp.tile([16, SIZE], F32, tag="tws", name="tws")
            nc.sync.dma_start(out=tws, in_=tw_d[:, :])
            et = pp.tile([16, SIZE], F32, tag="et", name="et")
            nc.scalar.activation(et, tws, AF.Exp)
            # fp32 PE matmuls only carry ~bf16 precision here, so split et
            # into bf16 hi+lo and accumulate two exact bf16 matmuls.
            c16b = pp.tile([16, 5], BF16, tag="c16b", name="c16b")
            nc.vector.tensor_copy(c16b, c16s)
            ethi = pp.tile([16, SIZE], BF16, tag="ethi", name="ethi")
            nc.vector.tensor_copy(ethi, et)
            etlo = pp.tile([16, SIZE], BF16, tag="etlo", name="etlo")
            nc.vector.scalar_tensor_tensor(
                etlo, et, 1.0, ethi, op0=OP.mult, op1=OP.subtract
            )
            psw = psp1.tile([128, MB, 5], F32, tag="psw", name="psw")
            for mb in range(MB):
                ms = slice(mb * 128, (mb + 1) * 128)
                nc.tensor.matmul(
                    psw[:, mb, :], ethi[:, ms], c16b[:, :], start=True, stop=False
                )
                nc.tensor.matmul(
                    psw[:, mb, :], etlo[:, ms], c16b[:, :], start=False, stop=True
                )

            # --- weights: exp in transposed layout + row sums ---
            eaT = [pp.tile([128, SIZE], BF16, tag=f"ea{kb}", name=f"ea{kb}") for kb in range(KB)]
            ebT = [pp.tile([128, SIZE], BF16, tag=f"eb{kb}", name=f"eb{kb}") for kb in range(KB)]
            pssa = psp1.tile([128, MB], F32, tag="pssa", name="pssa")
            pssb = psp1.tile([128, MB], F32, tag="pssb", name="pssb")
            for kb in range(KB):
                ks = slice(kb * 128, (kb + 1) * 128)
                wfa = wstage.tile([128, SIZE], F32, tag="wf32", name="wf32")
                nc.sync.dma_start(out=wfa, in_=wat_d[ks, :])
                nc.scalar.activation(eaT[kb], wfa, AF.Exp)
                wfb = wstage.tile([128, SIZE], F32, tag="wf32", name="wf32")
                nc.sync.dma_start(out=wfb, in_=wbt_d[ks, :])
                nc.scalar.activation(ebT[kb], wfb, AF.Exp)
            # mb-outer so each column's PSUM accumulation group is contiguous
            # in PE order — interleaved groups in one bank corrupt results.
            for mb in range(MB):
                ms = slice(mb * 128, (mb + 1) * 128)
                for kb in range(KB):
                    nc.tensor.matmul(
                        pssa[:, mb : mb + 1],
                        eaT[kb][:, ms],
                        ones[:, :],
                        start=(kb == 0),
                        stop=(kb == KB - 1),
                    )
                for kb in range(KB):
                    nc.tensor.matmul(
                        pssb[:, mb : mb + 1],
                        ebT[kb][:, ms],
                        ones[:, :],
                        start=(kb == 0),
                        stop=(kb == KB - 1),
                    )

            # --- assemble final coefficients [128, MB] ---
            sa = pp.tile([128, MB], F32, tag="sa", name="sa")
            nc.vector.tensor_copy(sa, pssa)
            sb = pp.tile([128, MB], F32, tag="sb", name="sb")
            nc.vector.tensor_copy(sb, pssb)
            ra = pp.tile([128, MB], F32, tag="ra", name="ra")
            nc.vector.reciprocal(ra, sa)
            rb = pp.tile([128, MB], F32, tag="rb", name="rb")
            nc.vector.reciprocal(rb, sb)
            wraw = pp.tile([128, MB, 5], F32, tag="wraw", name="wraw")
            nc.vector.tensor_copy(wraw, psw)
            rt = pp.tile([128, MB], F32, tag="rt", name="rt")
            nc.vector.reciprocal(rt, wraw[:, :, 4])
            tA = pp.tile([128, MB], F32, tag="tA", name="tA")
            nc.vector.tensor_mul(tA, rt, ra)
            tB = pp.tile([128, MB], F32, tag="tB", name="tB")
            nc.vector.tensor_mul(tB, rt, rb)
            tAB = pp.tile([128, MB], F32, tag="tAB", name="tAB")
            nc.vector.tensor_mul(tAB, tA, rb)
            w1f = pp.tile([128, MB], F32, tag="w1f", name="w1f")
            nc.vector.tensor_mul(w1f, wraw[:, :, 0], rt)
            wAf = pp.tile([128, MB], F32, tag="wAf", name="wAf")
            nc.vector.tensor_mul(wAf, wraw[:, :, 1], tA)
            wBf = pp.tile([128, MB], F32, tag="wBf", name="wBf")
            nc.vector.tensor_mul(wBf, wraw[:, :, 2], tB)
            wABf = pp.tile([128, MB], F32, tag="wABf", name="wABf")
            nc.vector.tensor_mul(wABf, wraw[:, :, 3], tAB)

            # --- main loop ---
            for _rep in range(reps):
              for nt in range(N_NT):
                ns = slice(nt * NT, (nt + 1) * NT)
                xb = []
                for kb in range(KB):
                    ks = slice(kb * 128, (kb + 1) * 128)
                    xf = xstage.tile([128, NT], F32, tag="xf", name="xf")
                    nc.sync.dma_start(out=xf, in_=x_d[ks, ns])
                    xbt = xbuf.tile([128, NT], BF16, tag=f"xb{kb}", name=f"xb{kb}")
                    nc.scalar.activation(xbt, xf, AF.Copy)
                    xb.append(xbt)
                for mb in range(MB):
                    ms = slice(mb * 128, (mb + 1) * 128)
                    pa = psp.tile([128, NT], F32, tag="pa", name="pa")
                    pb = psp.tile([128, NT], F32, tag="pb", name="pb")
                    for kb in range(KB):
                        nc.tensor.matmul(
                            pa,
                            eaT[kb][:, ms],
                            xb[kb][:, :],
                            start=(kb == 0),
                            stop=(kb == KB - 1),
                        )
                    for kb in range(KB):
                        nc.tensor.matmul(
                            pb,
                            ebT[kb][:, ms],
                            xb[kb][:, :],
                            start=(kb == 0),
                            stop=(kb == KB - 1),
                        )
                    # epilogue: out = (ta*wAB' + wA')*ta? -- no:
                    #   u = tb*wAB' + wA'          (DVE tensor_scalar dual-op)
                    #   v = tb*wB' + w1'           (ACT identity scale/bias)
                    #   w = ta*u                   (DVE)
                    #   o = w + v                  (GPSIMD, SBUF only)
                    u = epi.tile([128, NT], F32, tag="u", name="u")
                    nc.vector.tensor_scalar(
                        u,
                        pb,
                        wABf[:, mb : mb + 1],
                        wAf[:, mb : mb + 1],
                        op0=OP.mult,
                        op1=OP.add,
                    )
                    v = epi.tile([128, NT], F32, tag="v", name="v")
                    nc.scalar.activation(
                        v,
                        pb,
                        AF.Identity,
                        bias=w1f[:, mb : mb + 1],
                        scale=wBf[:, mb : mb + 1],
                    )
                    w = epi.tile([128, NT], F32, tag="w", name="w")
                    nc.vector.tensor_mul(w, pa, u)
                    o = outp.tile([128, NT], F32, tag="o", name="o")
                    nc.gpsimd.tensor_add(o, w, v)
                    nc.sync.dma_start(out=out_d[ms, ns], in_=o)

    _split_waits(nc)
    return nc


_NC_CACHE = None


def _get_nc():
    global _NC_CACHE
    if _NC_CACHE is None:
        _NC_CACHE = build_nc()
    return _NC_CACHE


def kernel(prev_layer_output, input_A_weights, input_B_weights, table_weights):
    x = np.ascontiguousarray(np.asarray(prev_layer_output, dtype=np.float32))
    wa = np.asarray(input_A_weights, dtype=np.float32)
    wb = np.asarray(input_B_weights, dtype=np.float32)
    tw = np.ascontiguousarray(np.asarray(table_weights, dtype=np.float32))
    wat = np.ascontiguousarray(wa.T)
    wbt = np.ascontiguousarray(wb.T)

    nc = _get_nc()
    in_maps = [
        {
            "x": np.ascontiguousarray(x[:, c * NB : (c + 1) * NB]),
            "wat": wat,
            "wbt": wbt,
            "tw": tw,
        }
        for c in range(N_CORES)
    ]
    res = run_bass_kernel_spmd(nc, in_maps, core_ids=list(range(N_CORES)))
    return np.concatenate([res.results[c]["out"] for c in range(N_CORES)], axis=1)



# revision 3
# speedup vs baseline: 5.4293x; 5.4293x over previous
"""Trainium2 Bass kernel for nn_LogicLayer (soft logic-gate layer).

Math (single core):
  pA = softmax(Wa, axis=1); pB = softmax(Wb, axis=1); pT = softmax(tw, axis=0)
  a = pA @ X ; b = pB @ X
  out = sum_g pT[g] * gate_g(a, b)

Each of the 16 soft gates is affine in {1, A, B, A*B}, so with C[g, :] =
(c1, cA, cB, cAB) per gate:
  out = w1 + wA*a + wB*b + wAB*(a*b),   w_j[m] = sum_g pT[g, m] * C[g, j]

All softmax normalizers fold into the coefficients: with unnormalized
Ea = exp(Wa) (no max-subtraction needed; Wa ~ N(0,1)), ta = Ea^T-matmul,
a = ta / sA[m], and pT = exp(tw)/sT:
  out = w1' + wA'*ta + wB'*tb + wAB'*ta*tb

Dispatch-dominated regime: on this axon-tunneled setup, per-execute cost is
  base (~2.4 ms) + ~0.14 ms/operand + input-bytes / ~15 GB/s,
while on-device exec is only ~0.4 ms. Hence: ONE core, ONE packed fp16
input tensor [1024, 18448] = x | Wa^T | Wb^T | tw^T (36 MiB -> half the
f32 bytes), no zero-output staging operands. fp16 carries 11 mantissa
bits so accuracy improves over the previous bf16 on-chip rounding.
"""

import sys

if "/opt/trn_rl_repo" not in sys.path:
    sys.path.insert(0, "/opt/trn_rl_repo")

import numpy as np

import concourse.bass as bass
import concourse.mybir as mybir
import concourse.tile as tile

SIZE = 1024
PREV = 1024
BATCH = 16384
NB = BATCH             # single core: full batch
NT = 512               # n-tile (one PSUM bank of f32)
N_NT = NB // NT        # 32
KB = PREV // 128       # 8 k-blocks
MB = SIZE // 128       # 8 m-blocks

F32 = mybir.dt.float32
F16 = mybir.dt.float16

# Packed input column offsets (fp16 [PREV, PKW])
XC0 = 0
WA0 = BATCH
WB0 = WA0 + SIZE
TW0 = WB0 + SIZE
PKW = TW0 + 16

# Quantize x to uint8 on host: shrinks the dominant marshal operand from
# 32 MiB fp16 to 16 MiB. Dequant on-chip via ACT copy with scale+bias.
USE_U8_X = False

# Gate coefficient matrix: columns = [const, A, B, AB, ones]; rows = gate id.
_C16 = np.array(
    [
        # 1   A   B  AB  ones
        [0,  0,  0,  0, 1],  # 0  FALSE
        [0,  0,  0,  1, 1],  # 1  A AND B
        [0,  1,  0, -1, 1],  # 2  A AND NOT B
        [0,  1,  0,  0, 1],  # 3  A
        [0,  0,  1, -1, 1],  # 4  NOT A AND B
        [0,  0,  1,  0, 1],  # 5  B
        [0,  1,  1, -2, 1],  # 6  XOR
        [0,  1,  1, -1, 1],  # 7  OR
        [1, -1, -1,  1, 1],  # 8  NOR
        [1, -1, -1,  2, 1],  # 9  XNOR
        [1,  0, -1,  0, 1],  # 10 NOT B
        [1,  0, -1,  1, 1],  # 11 B -> A
        [1, -1,  0,  0, 1],  # 12 NOT A
        [1, -1,  0,  1, 1],  # 13 A -> B
        [1,  0,  0, -1, 1],  # 14 NAND
        [1,  0,  0,  0, 1],  # 15 TRUE
    ],
    dtype=np.float32,
)


def _split_waits(nc, maxw=1):
    """Walrus in this container encodes at most one sync-wait per
    instruction; hoist excess waits into preceding NoOps on the same
    engine (semantically an AND of waits, executed in sequence)."""
    for f in nc.m.functions:
        for blk in f.blocks:
            new_list = []
            changed = False
            for inst in blk.instructions:
                si = inst.sync_info
                if si is not None and len(si.on_wait) > maxw:
                    waits = list(si.on_wait)
                    chunks = [waits[i : i + maxw] for i in range(0, len(waits), maxw)]
                    for ci, ch in enumerate(chunks[:-1]):
                        nop = mybir.InstNoOp(
                            name=f"{inst.name}-wsplit{ci}", ins=[], outs=[]
                        )
                        nop.engine = inst.engine
                        nop.sync_info = mybir.SyncInfo(on_wait=ch, on_update=[])
                        new_list.append(nop)
                    inst.sync_info = mybir.SyncInfo(
                        on_wait=chunks[-1], on_update=list(si.on_update)
                    )
                    changed = True
                new_list.append(inst)
            if changed:
                blk.instructions = new_list


def build_nc(reps=1):
    # reps>1 repeats the main loop inside the NEFF (timing only: slope
    # between two reps values isolates steady-state main-loop time from
    # the axon dispatch floor).
    nc = bass.Bass()
    if USE_U8_X:
        x_d = nc.dram_tensor("xq", [PREV, NB], mybir.dt.uint8, kind="ExternalInput")
        w_d = nc.dram_tensor("wpk", [PREV, 2 * SIZE + 16], F16, kind="ExternalInput")
        wa0, wb0, tw0 = 0, SIZE, 2 * SIZE
    else:
        pk_d = nc.dram_tensor("pk", [PREV, PKW], F16, kind="ExternalInput")
        x_d, w_d = pk_d, pk_d
        wa0, wb0, tw0 = WA0, WB0, TW0
    out_d = nc.dram_tensor("out", [SIZE, NB], F32, kind="ExternalOutput")
    c16_d = nc.inline_tensor(_C16, "c16")

    AF = mybir.ActivationFunctionType
    OP = mybir.AluOpType

    with tile.TileContext(nc) as tc:
        with (
            tc.tile_pool(name="persist", bufs=1) as pp,
            tc.tile_pool(name="wstage", bufs=3) as wstage,
            tc.tile_pool(name="xstage", bufs=6) as xstage,
            tc.tile_pool(name="xbuf", bufs=2) as xbuf,
            tc.tile_pool(name="epi", bufs=3) as epi,
            tc.tile_pool(name="outp", bufs=4) as outp,
            tc.tile_pool(name="psum", bufs=2, space="PSUM") as psp,
            tc.tile_pool(name="psum1", bufs=1, space="PSUM") as psp1,
        ):
            # --- constants ---
            c16s = pp.tile([16, 5], F32, tag="c16s", name="c16s")
            nc.sync.dma_start(out=c16s, in_=c16_d[:, :])
            c16h = pp.tile([16, 5], F16, tag="c16h", name="c16h")
            nc.vector.tensor_copy(c16h, c16s)
            ones = pp.tile([128, 1], F16, tag="ones", name="ones")
            nc.vector.memset(ones, 1.0)

            # --- table coefficients (tw stored transposed: [PREV, 16]) ---
            twt = pp.tile([16, SIZE], F16, tag="twt", name="twt")
            nc.sync.dma_start(
                out=twt, in_=w_d[:, tw0 : tw0 + 16].rearrange("a b -> b a")
            )
            et = pp.tile([16, SIZE], F32, tag="et", name="et")
            nc.scalar.activation(et, twt, AF.Exp)
            # fp16 PE matmuls carry ~11-bit precision; split et into fp16
            # hi+lo and accumulate two exact fp16 matmuls.
            ethi = pp.tile([16, SIZE], F16, tag="ethi", name="ethi")
            nc.vector.tensor_copy(ethi, et)
            etlo = pp.tile([16, SIZE], F16, tag="etlo", name="etlo")
            nc.vector.scalar_tensor_tensor(
                etlo, et, 1.0, ethi, op0=OP.mult, op1=OP.subtract
            )
            psw = psp1.tile([128, MB, 5], F32, tag="psw", name="psw")
            for mb in range(MB):
                ms = slice(mb * 128, (mb + 1) * 128)
                nc.tensor.matmul(
                    psw[:, mb, :], ethi[:, ms], c16h[:, :], start=True, stop=False
                )
                nc.tensor.matmul(
                    psw[:, mb, :], etlo[:, ms], c16h[:, :], start=False, stop=True
                )

            # --- weights: exp in transposed layout + row sums ---
            eaT = [pp.tile([128, SIZE], F16, tag=f"ea{kb}", name=f"ea{kb}") for kb in range(KB)]
            ebT = [pp.tile([128, SIZE], F16, tag=f"eb{kb}", name=f"eb{kb}") for kb in range(KB)]
            pssa = psp1.tile([128, MB], F32, tag="pssa", name="pssa")
            pssb = psp1.tile([128, MB], F32, tag="pssb", name="pssb")
            for kb in range(KB):
                ks = slice(kb * 128, (kb + 1) * 128)
                wfa = wstage.tile([128, SIZE], F16, tag="wf16", name="wf16")
                nc.sync.dma_start(out=wfa, in_=w_d[ks, wa0 : wa0 + SIZE])
                nc.scalar.activation(eaT[kb], wfa, AF.Exp)
                wfb = wstage.tile([128, SIZE], F16, tag="wf16", name="wf16")
                nc.sync.dma_start(out=wfb, in_=w_d[ks, wb0 : wb0 + SIZE])
                nc.scalar.activation(ebT[kb], wfb, AF.Exp)
            # mb-outer so each column's PSUM accumulation group is contiguous
            # in PE order — interleaved groups in one bank corrupt results.
            for mb in range(MB):
                ms = slice(mb * 128, (mb + 1) * 128)
                for kb in range(KB):
                    nc.tensor.matmul(
                        pssa[:, mb : mb + 1],
                        eaT[kb][:, ms],
                        ones[:, :],
                        start=(kb == 0),
                        stop=(kb == KB - 1),
                    )
                for kb in range(KB):
                    nc.tensor.matmul(
                        pssb[:, mb : mb + 1],
                        ebT[kb][:, ms],
                        ones[:, :],
                        start=(kb == 0),
                        stop=(kb == KB - 1),
                    )

            # --- assemble final coefficients [128, MB] ---
            sa = pp.tile([128, MB], F32, tag="sa", name="sa")
            nc.vector.tensor_copy(sa, pssa)
            sb = pp.tile([128, MB], F32, tag="sb", name="sb")
            nc.vector.tensor_copy(sb, pssb)
            ra = pp.tile([128, MB], F32, tag="ra", name="ra")
            nc.vector.reciprocal(ra, sa)
            rb = pp.tile([128, MB], F32, tag="rb", name="rb")
            nc.vector.reciprocal(rb, sb)
            wraw = pp.tile([128, MB, 5], F32, tag="wraw", name="wraw")
            nc.vector.tensor_copy(wraw, psw)
            rt = pp.tile([128, MB], F32, tag="rt", name="rt")
            nc.vector.reciprocal(rt, wraw[:, :, 4])
            tA = pp.tile([128, MB], F32, tag="tA", name="tA")
            nc.vector.tensor_mul(tA, rt, ra)
            tB = pp.tile([128, MB], F32, tag="tB", name="tB")
            nc.vector.tensor_mul(tB, rt, rb)
            tAB = pp.tile([128, MB], F32, tag="tAB", name="tAB")
            nc.vector.tensor_mul(tAB, tA, rb)
            w1f = pp.tile([128, MB], F32, tag="w1f", name="w1f")
            nc.vector.tensor_mul(w1f, wraw[:, :, 0], rt)
            wAf = pp.tile([128, MB], F32, tag="wAf", name="wAf")
            nc.vector.tensor_mul(wAf, wraw[:, :, 1], tA)
            wBf = pp.tile([128, MB], F32, tag="wBf", name="wBf")
            nc.vector.tensor_mul(wBf, wraw[:, :, 2], tB)
            wABf = pp.tile([128, MB], F32, tag="wABf", name="wABf")
            nc.vector.tensor_mul(wABf, wraw[:, :, 3], tAB)

            # --- main loop ---
            for _rep in range(reps):
              for nt in range(N_NT):
                ns = slice(nt * NT, (nt + 1) * NT)
                xb = []
                for kb in range(KB):
                    ks = slice(kb * 128, (kb + 1) * 128)
                    if USE_U8_X:
                        xq = xstage.tile([128, NT], mybir.dt.uint8, tag="xq", name="xq")
                        nc.sync.dma_start(out=xq, in_=x_d[ks, ns])
                        xbt = xbuf.tile([128, NT], F16, tag=f"xb{kb}", name=f"xb{kb}")
                        # dequant: x = (k + 0.5) / 256
                        nc.scalar.activation(
                            xbt, xq, AF.Copy, bias=1.0 / 512, scale=1.0 / 256
                        )
                    else:
                        xbt = xbuf.tile([128, NT], F16, tag=f"xb{kb}", name=f"xb{kb}")
                        nc.sync.dma_start(out=xbt, in_=x_d[ks, ns])
                    xb.append(xbt)
                for mb in range(MB):
                    ms = slice(mb * 128, (mb + 1) * 128)
                    pa = psp.tile([128, NT], F32, tag="pa", name="pa")
                    pb = psp.tile([128, NT], F32, tag="pb", name="pb")
                    for kb in range(KB):
                        nc.tensor.matmul(
                            pa,
                            eaT[kb][:, ms],
                            xb[kb][:, :],
                            start=(kb == 0),
                            stop=(kb == KB - 1),
                        )
                    for kb in range(KB):
                        nc.tensor.matmul(
                            pb,
                            ebT[kb][:, ms],
                            xb[kb][:, :],
                            start=(kb == 0),
                            stop=(kb == KB - 1),
                        )
                    # epilogue:
                    #   u = tb*wAB' + wA'          (DVE tensor_scalar dual-op)
                    #   v = tb*wB' + w1'           (ACT identity scale/bias)
                    #   w = ta*u                   (DVE)
                    #   o = w + v                  (GPSIMD, SBUF only)
                    u = epi.tile([128, NT], F32, tag="u", name="u")
                    nc.vector.tensor_scalar(
                        u,
                        pb,
                        wABf[:, mb : mb + 1],
                        wAf[:, mb : mb + 1],
                        op0=OP.mult,
                        op1=OP.add,
                    )
                    v = epi.tile([128, NT], F32, tag="v", name="v")
                    nc.scalar.activation(
                        v,
                        pb,
                        AF.Identity,
                        bias=w1f[:, mb : mb + 1],
                        scale=wBf[:, mb : mb + 1],
                    )
                    w = epi.tile([128, NT], F32, tag="w", name="w")
                    nc.vector.tensor_mul(w, pa, u)
                    o = outp.tile([128, NT], F32, tag="o", name="o")
                    nc.gpsimd.tensor_add(o, w, v)
                    nc.sync.dma_start(out=out_d[ms, ns], in_=o)

    _split_waits(nc)
    return nc


_NC_CACHE = None
_EXEC_CACHE = None


def _get_nc():
    global _NC_CACHE
    if _NC_CACHE is None:
        _NC_CACHE = build_nc()
    return _NC_CACHE


def make_exec(nc):
    """Single-core executable for `nc`: inputs only (no zero-output staging
    operands — the kernel writes every output element), plain jit on
    device 0."""
    import jax
    import concourse.bass2jax as b2j

    b2j.install_neuronx_cc_hook()

    part_name = nc.partition_id_tensor.name if nc.partition_id_tensor else None
    in_names, out_names, out_avals = [], [], []
    for alloc in nc.m.functions[0].allocations:
        if not isinstance(alloc, mybir.MemoryLocationSet):
            continue
        name = alloc.memorylocations[0].name
        if alloc.kind == "ExternalInput":
            if name != part_name:
                in_names.append(name)
        elif alloc.kind == "ExternalOutput":
            out_names.append(name)
            out_avals.append(
                jax.core.ShapedArray(
                    tuple(alloc.tensor_shape), mybir.dt.np(alloc.dtype)
                )
            )
    all_in_names = list(in_names)
    if part_name is not None:
        all_in_names.append(part_name)

    def _body(*args):
        operands = list(args)
        if part_name is not None:
            operands.append(b2j.partition_id_tensor())
        outs = b2j._bass_exec_p.bind(
            *operands,
            out_avals=tuple(out_avals),
            in_names=tuple(all_in_names),
            out_names=tuple(out_names),
            lowering_input_output_aliases=(),
            sim_require_finite=True,
            sim_require_nnan=True,
            nc=nc,
        )
        return tuple(outs)

    return jax.jit(_body), in_names


def pack_inputs(prev_layer_output, input_A_weights, input_B_weights, table_weights):
    x = np.asarray(prev_layer_output, dtype=np.float32)
    wa = np.asarray(input_A_weights, dtype=np.float32)
    wb = np.asarray(input_B_weights, dtype=np.float32)
    tw = np.asarray(table_weights, dtype=np.float32)
    if USE_U8_X:
        xq = np.clip(np.floor(x * 256.0), 0, 255).astype(np.uint8)
        wpk = np.empty((PREV, 2 * SIZE + 16), np.float16)
        wpk[:, 0:SIZE] = wa.T
        wpk[:, SIZE : 2 * SIZE] = wb.T
        wpk[:, 2 * SIZE :] = tw.T
        return {"xq": np.ascontiguousarray(xq), "wpk": wpk}
    pk = np.empty((PREV, PKW), np.float16)
    pk[:, XC0:WA0] = x
    pk[:, WA0:WB0] = wa.T
    pk[:, WB0:TW0] = wb.T
    pk[:, TW0:PKW] = tw.T
    return {"pk": pk}


def kernel(prev_layer_output, input_A_weights, input_B_weights, table_weights):
    global _EXEC_CACHE
    import jax

    if _EXEC_CACHE is None:
        _EXEC_CACHE = make_exec(_get_nc())
    run, in_names = _EXEC_CACHE

    inp = pack_inputs(
        prev_layer_output, input_A_weights, input_B_weights, table_weights
    )
    dev = jax.devices()[0]
    args = [jax.device_put(inp[nm], dev) for nm in in_names]
    out = run(*args)
    return np.asarray(out[0])
